# revision 5
# baseline (speedup 1.0000x reference)
"""Trainium2 Bass kernel for nn_CNN3DRNN (3D-CNN over sliding windows + stacked GRU).

Strategy:
  - 26 (batch, window) CNN items sharded over 8 cores (4 slots/core, zero-padded).
  - Conv3D as tap-packed float32r matmuls (channel contraction on partitions) over
    zero-padded activation planes; kd taps packed into the partition (K) dim via
    duplicated partition groups, kh/kw taps via AP offsets.
  - Streaming per-frame pipeline with ring buffers; evictions (ACT) write padded
    interiors at arbitrary partition offsets.
  - Per-item GAP features AllGathered across cores; the stacked GRU + heads run
    redundantly on every core; core 0's output is returned.
"""

import numpy as np

import concourse.bass as bass
import concourse.mybir as mybir
from concourse import bacc
from concourse import bass_utils
from concourse.tile import TileContext
from concourse.masks import make_identity

F32 = mybir.dt.float32
F32R = mybir.dt.float32r
AF = mybir.ActivationFunctionType

B, T, WIN, NEWT = 2, 20, 8, 13
N_ITEMS = B * NEWT  # 26
N_CORES = 8
SLOTS = 4
OFFS = [0, 4, 8, 11, 14, 17, 20, 23]
CNT = [4, 4, 3, 3, 3, 3, 3, 3]
GAP_N = float(WIN * 14 * 14)  # 1568

HP56, FR56 = 58, 58 * 58   # 3364
HP28, FR28 = 30, 30 * 30   # 900
HP14, FR14 = 16, 16 * 16   # 256

TAPS9 = [(kh, kw) for kh in range(3) for kw in range(3)]
TAPS27 = [(kd, kh, kw) for kd in range(3) for kh in range(3) for kw in range(3)]


def _item_list():
    return [(g // NEWT, g % NEWT) for g in range(N_ITEMS)]


def _build_im2col(win):
    """win: (8, 56, 56, 3) -> (8, 81, 3364): rows (kd,kh,kw,c), padded 58x58."""
    xpad = np.zeros((10, 60, 60, 3), np.float32)
    xpad[1:9, 2:58, 2:58, :] = win
    out = np.empty((8, 81, 58, 58), np.float32)
    r = 0
    for kd, kh, kw in TAPS27:
        for c in range(3):
            out[:, r] = xpad[kd:kd + 8, kh:kh + 58, kw:kw + 58, c]
            r += 1
    return out.reshape(8, 81, FR56)


def _pack_weights(inp):
    w = {}
    w["wc11"] = np.asarray(inp["c11_k"], np.float32).reshape(81, 32)

    def dup3_pack(kk, cout):
        kk = np.asarray(kk, np.float32)
        cin = kk.shape[3]
        out = np.empty((3 * cin, 9 * cout), np.float32)
        for kh, kw in TAPS9:
            j = kh * 3 + kw
            out[:, j * cout:(j + 1) * cout] = kk[:, kh, kw].reshape(3 * cin, cout)
        return out

    def dup2_pack(kk, cout):
        kk = np.asarray(kk, np.float32)
        cin = kk.shape[3]
        wa = np.empty((2 * cin, 9 * cout), np.float32)
        wb = np.empty((cin, 9 * cout), np.float32)
        for kh, kw in TAPS9:
            j = kh * 3 + kw
            wa[:, j * cout:(j + 1) * cout] = kk[0:2, kh, kw].reshape(2 * cin, cout)
            wb[:, j * cout:(j + 1) * cout] = kk[2, kh, kw]
        return wa, wb

    w["wc12"] = dup3_pack(inp["c12_k"], 32)
    w["wc21"] = dup3_pack(inp["c21_k"], 64)
    w["wc22a"], w["wc22b"] = dup2_pack(inp["c22_k"], 64)
    w["wc31a"], w["wc31b"] = dup2_pack(inp["c31_k"], 128)

    k32 = np.asarray(inp["c32_k"], np.float32)
    wc32 = np.empty((128, 27 * 128), np.float32)
    for m, (kd, kh, kw) in enumerate(TAPS27):
        wc32[:, m * 128:(m + 1) * 128] = k32[kd, kh, kw]
    w["wc32"] = wc32

    cb = np.zeros((128, 6), np.float32)
    for i, nm in enumerate(["c11", "c12", "c21", "c22", "c31", "c32"]):
        b = np.asarray(inp[nm + "_b"], np.float32)
        cb[:len(b), i] = b
    w["convb"] = cb

    w["wg1x"] = np.asarray(inp["g1_wx"], np.float32) / GAP_N
    w["wg2x"] = np.asarray(inp["g2_wx"], np.float32)
    w["wg3x"] = np.asarray(inp["g3_wx"], np.float32)
    w["wg1h"] = np.asarray(inp["g1_wh"], np.float32)
    w["wg2h"] = np.asarray(inp["g2_wh"], np.float32)
    w["wg3h"] = np.asarray(inp["g3_wh"], np.float32)
    gb = np.zeros((64, 12), np.float32)
    for li, nm in enumerate(["g1", "g2", "g3"]):
        b = np.asarray(inp[nm + "_b"], np.float32)
        gb[:, li * 4 + 0] = b[0, 0:64] + b[1, 0:64]
        gb[:, li * 4 + 1] = b[0, 64:128] + b[1, 64:128]
        gb[:, li * 4 + 2] = b[1, 128:192]
        gb[:, li * 4 + 3] = b[0, 128:192]
    w["grub"] = gb

    w["d1w"] = np.asarray(inp["d1_w"], np.float32)
    w["d2w"] = np.asarray(inp["d2_w"], np.float32)
    w["thrw"] = np.asarray(inp["thr_w"], np.float32)
    w["toriw"] = np.asarray(inp["tori_w"], np.float32)
    hb = np.zeros((64, 4), np.float32)
    hb[0:64, 0] = np.asarray(inp["d1_b"], np.float32)
    hb[0:32, 1] = np.asarray(inp["d2_b"], np.float32)
    hb[0:64, 2] = np.asarray(inp["thr_b"], np.float32)
    hb[0:2, 3] = np.asarray(inp["tori_b"], np.float32)
    w["headb"] = hb
    return w


WEIGHT_SPECS = [
    ("wc11", [81, 32], F32R), ("wc12", [96, 9 * 32], F32R),
    ("wc21", [96, 9 * 64], F32R),
    ("wc22a", [128, 9 * 64], F32R), ("wc22b", [64, 9 * 64], F32R),
    ("wc31a", [128, 9 * 128], F32R), ("wc31b", [64, 9 * 128], F32R),
    ("wc32", [128, 27 * 128], F32R),
    ("convb", [128, 6], F32),
    ("wg1x", [128, 192], F32), ("wg2x", [64, 192], F32), ("wg3x", [64, 192], F32),
    ("wg1h", [64, 192], F32), ("wg2h", [64, 192], F32), ("wg3h", [64, 192], F32),
    ("grub", [64, 12], F32),
    ("d1w", [64, 64], F32), ("d2w", [64, 32], F32),
    ("thrw", [32, 64], F32), ("toriw", [32, 2], F32), ("headb", [64, 4], F32),
]


def _segments():
    segs = []
    for c in range(N_CORES):
        g0, n = OFFS[c], CNT[c]
        if g0 < NEWT < g0 + n:
            segs.append((c, 0, NEWT - g0, g0))
            segs.append((c, NEWT - g0, g0 + n - NEWT, NEWT))
        else:
            segs.append((c, 0, n, g0))
    return segs


def _dstcol(g):
    return (g % NEWT) * 2 + (g // NEWT)


def build_program():
    nc = bacc.Bacc()
    x_d = nc.dram_tensor("x_im2col", [SLOTS, 8, 81, FR56], F32R,
                         kind="ExternalInput")
    wd = {}
    for nm, shape, dt in WEIGHT_SPECS:
        wd[nm] = nc.dram_tensor(nm, shape, dt, kind="ExternalInput")
    throw_d = nc.dram_tensor("throw", [2, 64], F32, kind="ExternalOutput")
    tori_d = nc.dram_tensor("tori", [2, 2], F32, kind="ExternalOutput")
    feats_sh = nc.dram_tensor("feats_sh", [1, 128 * SLOTS], F32)
    feats_gd = nc.dram_tensor("feats_gd", [1, 128 * SLOTS * N_CORES], F32,
                              addr_space="Shared")

    with TileContext(nc) as tc:
        with (
            tc.tile_pool(name="wpool", bufs=1) as wpool,
            tc.tile_pool(name="persist", bufs=1) as pp,
            tc.tile_pool(name="small", bufs=2) as sp,
            tc.tile_pool(name="cnnps", bufs=4, space="PSUM") as pspool,
            tc.tile_pool(name="grups", bufs=2, space="PSUM") as pspool2,
        ):
            wt = {}
            for nm, shape, dt in WEIGHT_SPECS:
                if nm in ("wc22b", "wc31b"):
                    # lhsT must share base_partition with its rhs (64)
                    full = wpool.tile([128, shape[1]], dt, tag=nm, name=nm)
                    nc.sync.dma_start(full[64:128, :], wd[nm][:])
                    wt[nm] = full
                else:
                    wt[nm] = wpool.tile(shape, dt, tag=nm, name=nm)
                    nc.sync.dma_start(wt[nm][:], wd[nm][:])
            ident = wpool.tile([64, 64], F32, tag="ident")
            make_identity(nc, ident[:])

            c11in = [pp.tile([81, FR56], F32R, tag=f"c11in{i}", name=f"c11in{i}") for i in range(2)]
            c12ch = [pp.tile([96, FR56], F32R, tag=f"c12ch{i}", name=f"c12ch{i}") for i in range(3)]
            c12out = [pp.tile([32, 3136], F32R, tag=f"c12o{i}", name=f"c12o{i}") for i in range(2)]
            p1tmp = [pp.tile([32, 1568], F32R, tag=f"p1t{i}", name=f"p1t{i}") for i in range(2)]
            c21ch = [pp.tile([96, FR28], F32R, tag=f"c21ch{i}", name=f"c21ch{i}") for i in range(3)]
            c22ch = [pp.tile([128, FR28], F32R, tag=f"c22ch{i}", name=f"c22ch{i}") for i in range(3)]
            c22out = [pp.tile([64, 784], F32R, tag=f"c22o{i}", name=f"c22o{i}") for i in range(2)]
            p2tmp = [pp.tile([64, 392], F32R, tag=f"p2t{i}", name=f"p2t{i}") for i in range(2)]
            c31in = pp.tile([128, 9 * FR14], F32R, tag="c31in")
            c32in = pp.tile([128, 10 * FR14], F32R, tag="c32in")
            evs = [pp.tile([128, 392], F32R, tag=f"evs{i}", name=f"evs{i}") for i in range(2)]
            gapac = pp.tile([128, 4], F32, tag="gapac")
            feats = pp.tile([128, SLOTS], F32, tag="feats")

            for t_ in (c11in + c12ch + c12out + p1tmp + c21ch + c22ch + c22out
                       + p2tmp + [c31in, c32in]):
                nc.vector.memset(t_[:].bitcast(F32), 0.0)

            cb = wt["convb"]

            # ---------- per-stage emitters -----------------------------------
            def st_c11(item, f):
                xin = c11in[f % 2]
                nc.sync.dma_start(xin[:], x_d[item, f])
                ch = c12ch[f % 3]
                dst = ch[32:64, :].rearrange("k (h w) -> k h w", h=HP56)
                xv = xin[:].rearrange("k (h w) -> k h w", h=HP56)
                for tr in range(7):
                    ps = pspool.tile([128, 448], F32, tag="ps")
                    nc.tensor.matmul(
                        ps[0:32, :], wt["wc11"][:],
                        xv[:, 1 + tr * 8:9 + tr * 8, 1:57],
                        start=True, stop=True)
                    nc.scalar.activation(
                        dst[:, 1 + tr * 8:9 + tr * 8, 1:57],
                        ps[0:32, :].rearrange("c (h w) -> c h w", h=8),
                        AF.Relu, bias=cb[0:32, 0:1])
                if f + 1 <= 7:
                    nc.sync.dma_start(c12ch[(f + 1) % 3][0:32, :], ch[32:64, :])
                if f - 1 >= 0:
                    nc.sync.dma_start(c12ch[(f - 1) % 3][64:96, :], ch[32:64, :])

            def st_c12(item, d):
                chv = c12ch[d % 3][:].rearrange("k (h w) -> k h w", h=HP56)
                co = c12out[d % 2]
                for tr in range(7):
                    ps = pspool.tile([128, 448], F32, tag="ps")
                    for j, (kh, kw) in enumerate(TAPS9):
                        nc.tensor.matmul(
                            ps[0:32, :], wt["wc12"][:, j * 32:(j + 1) * 32],
                            chv[:, tr * 8 + kh: tr * 8 + kh + 8, kw:kw + 56],
                            start=(j == 0), stop=(j == 8))
                    nc.scalar.activation(
                        co[:, tr * 448:(tr + 1) * 448], ps[0:32, :],
                        AF.Relu, bias=cb[0:32, 1:2])
                # pool1(d) -> c21 chunk group1 (frame d)
                pt = p1tmp[d % 2]
                cov = co[:].rearrange("c (h wo wi) -> c h wo wi", h=56, wi=2)
                ptv = pt[:].rearrange("c (h w) -> c h w", h=56)
                nc.vector.tensor_max(ptv[:], cov[:, :, :, 0], cov[:, :, :, 1])
                ptv2 = pt[:].rearrange("c (hp hh w) -> c hp hh w", hp=28, hh=2)
                dstp = c21ch[d % 3][32:64, :].rearrange("c (h w) -> c h w", h=HP28)
                nc.vector.tensor_max(dstp[:, 1:29, 1:29],
                                     ptv2[:, :, 0, :], ptv2[:, :, 1, :])
                if d + 1 <= 7:
                    nc.sync.dma_start(c21ch[(d + 1) % 3][0:32, :],
                                      c21ch[d % 3][32:64, :])
                if d - 1 >= 0:
                    nc.sync.dma_start(c21ch[(d - 1) % 3][64:96, :],
                                      c21ch[d % 3][32:64, :])

            def st_c21(item, d):
                chv = c21ch[d % 3][:].rearrange("k (h w) -> k h w", h=HP28)
                dst = c22ch[d % 3][64:128, :].rearrange("k (h w) -> k h w", h=HP28)
                for tr in range(2):
                    ps = pspool.tile([128, 392], F32, tag="ps")
                    for j, (kh, kw) in enumerate(TAPS9):
                        nc.tensor.matmul(
                            ps[0:64, :], wt["wc21"][:, j * 64:(j + 1) * 64],
                            chv[:, tr * 14 + kh: tr * 14 + kh + 14, kw:kw + 28],
                            start=(j == 0), stop=(j == 8))
                    nc.scalar.activation(
                        dst[:, 1 + tr * 14:15 + tr * 14, 1:29],
                        ps[0:64, :].rearrange("c (h w) -> c h w", h=14),
                        AF.Relu, bias=cb[0:64, 2:3])
                if d + 1 <= 7:
                    nc.sync.dma_start(c22ch[(d + 1) % 3][0:64, :],
                                      c22ch[d % 3][64:128, :])

            def st_c22(item, d):
                v_a = c22ch[d % 3][:].rearrange("k (h w) -> k h w", h=HP28)
                v_b = c22ch[(d + 1) % 3][64:128, :].rearrange(
                    "k (h w) -> k h w", h=HP28)
                co2 = c22out[d % 2]
                for tr in range(2):
                    ps = pspool.tile([128, 392], F32, tag="ps")
                    for j, (kh, kw) in enumerate(TAPS9):
                        nc.tensor.matmul(
                            ps[0:64, :], wt["wc22a"][:, j * 64:(j + 1) * 64],
                            v_a[:, tr * 14 + kh: tr * 14 + kh + 14, kw:kw + 28],
                            start=(j == 0), stop=False)
                    for j, (kh, kw) in enumerate(TAPS9):
                        nc.tensor.matmul(
                            ps[0:64, :], wt["wc22b"][64:128, j * 64:(j + 1) * 64],
                            v_b[:, tr * 14 + kh: tr * 14 + kh + 14, kw:kw + 28],
                            start=False, stop=(j == 8))
                    nc.scalar.activation(
                        co2[:, tr * 392:(tr + 1) * 392], ps[0:64, :],
                        AF.Relu, bias=cb[0:64, 3:4])
                # pool2(d) -> c31in top half, slot d (frame d)
                pt2 = p2tmp[d % 2]
                cov2 = co2[:].rearrange("c (h wo wi) -> c h wo wi", h=28, wi=2)
                pt2v = pt2[:].rearrange("c (h w) -> c h w", h=28)
                nc.vector.tensor_max(pt2v[:], cov2[:, :, :, 0], cov2[:, :, :, 1])
                pt2v2 = pt2[:].rearrange("c (hp hh w) -> c hp hh w", hp=14, hh=2)
                dst31 = c31in[64:128, :].rearrange("k (s h w) -> k s h w",
                                                   s=9, h=HP14)
                nc.vector.tensor_max(dst31[:, d, 1:15, 1:15],
                                     pt2v2[:, :, 0, :], pt2v2[:, :, 1, :])
                nc.sync.dma_start(c31in[0:64, (d + 1) * FR14:(d + 2) * FR14],
                                  c31in[64:128, d * FR14:(d + 1) * FR14])

            def st_c31(item, q):
                v31a = c31in[:].rearrange("k (s h w) -> k s h w", s=9, h=HP14)
                v31b = c31in[64:128, :].rearrange("k (s h w) -> k s h w",
                                                  s=9, h=HP14)
                ps = pspool.tile([128, 392], F32, tag="ps")
                for j, (kh, kw) in enumerate(TAPS9):
                    nc.tensor.matmul(
                        ps[:], wt["wc31a"][:, j * 128:(j + 1) * 128],
                        v31a[:, 2 * q:2 * q + 2, kh:kh + 14, kw:kw + 14],
                        start=(j == 0), stop=False)
                for j, (kh, kw) in enumerate(TAPS9):
                    nc.tensor.matmul(
                        ps[:], wt["wc31b"][64:128, j * 128:(j + 1) * 128],
                        v31b[:, 2 * q + 1:2 * q + 3, kh:kh + 14, kw:kw + 14],
                        start=False, stop=(j == 8))
                dst32 = c32in[:].rearrange("k (s h w) -> k s h w", s=10, h=HP14)
                nc.scalar.activation(
                    dst32[:, 2 * q + 1:2 * q + 3, 1:15, 1:15],
                    ps[:].rearrange("c (s h w) -> c s h w", s=2, h=14),
                    AF.Relu, bias=cb[:, 4:5])

            def st_c32(item, q):
                v32 = c32in[:].rearrange("k (s h w) -> k s h w", s=10, h=HP14)
                ps = pspool.tile([128, 392], F32, tag="ps")
                for m, (kd, kh, kw) in enumerate(TAPS27):
                    nc.tensor.matmul(
                        ps[:], wt["wc32"][:, m * 128:(m + 1) * 128],
                        v32[:, 2 * q + kd:2 * q + kd + 2, kh:kh + 14, kw:kw + 14],
                        start=(m == 0), stop=(m == 26))
                nc.scalar.activation(evs[q % 2][:], ps[:], AF.Relu,
                                     bias=cb[:, 5:6],
                                     accum_out=gapac[:, q:q + 1])

            # ---------- CNN: staged pipeline over frames ----------------------
            for item in range(SLOTS):
                for f in range(12):
                    if f == 0:
                        nc.vector.memset(c12ch[0][0:32, :].bitcast(F32), 0.0)
                    if f == 1:
                        nc.vector.memset(c21ch[0][0:32, :].bitcast(F32), 0.0)
                    if f == 2:
                        nc.vector.memset(c22ch[0][0:64, :].bitcast(F32), 0.0)
                    if f == 6:
                        nc.vector.memset(c12ch[1][64:96, :].bitcast(F32), 0.0)
                    if f == 7:
                        nc.vector.memset(c21ch[1][64:96, :].bitcast(F32), 0.0)
                    if f == 9:
                        nc.vector.memset(c22ch[2][64:128, :].bitcast(F32), 0.0)
                    if f <= 7:
                        st_c11(item, f)
                    if 0 <= f - 1 <= 7:
                        st_c12(item, f - 1)
                    if 0 <= f - 2 <= 7:
                        st_c21(item, f - 2)
                    if 0 <= f - 3 <= 7:
                        st_c22(item, f - 3)
                    if f in (5, 7, 9, 11):
                        st_c31(item, (f - 5) // 2)
                    if f == 7:
                        st_c32(item, 0)
                    if f == 9:
                        st_c32(item, 1)
                    if f == 11:
                        st_c32(item, 2)
                        st_c32(item, 3)
                nc.vector.reduce_sum(feats[:, item:item + 1], gapac[:],
                                     axis=mybir.AxisListType.X)

            # ---------- gather features across cores --------------------------
            nc.sync.dma_start(
                feats_sh[:].rearrange("o (p i) -> (o p) i", p=128), feats[:])
            nc.gpsimd.collective_compute(
                "AllGather", mybir.AluOpType.bypass,
                ins=[feats_sh.ap().opt()], outs=[feats_gd.ap().opt()],
                replica_groups=[list(range(N_CORES))],
            )
            ford = pp.tile([128, 2 * NEWT], F32, tag="ford")
            gd3 = feats_gd.ap().rearrange("o (c p i) -> (o c) p i",
                                          c=N_CORES, p=128)
            for (c, i0, n, g0) in _segments():
                nc.sync.dma_start(
                    ford[:, _dstcol(g0):_dstcol(g0) + 2 * n - 1:2],
                    gd3[c, :, i0:i0 + n])

            # ---------- stacked GRU (redundant on every core) -----------------
            gb = wt["grub"]
            seq_in = ford[:, 0:2 * NEWT]
            hT = None
            for li, (wxn, whn) in enumerate(
                    [("wg1x", "wg1h"), ("wg2x", "wg2h"), ("wg3x", "wg3h")]):
                wx, wh = wt[wxn], wt[whn]
                nx = 2 * NEWT
                xp = pp.tile([64, 3 * nx], F32, tag=f"xp{li}")
                for g in range(3):
                    psx = pspool2.tile([64, nx], F32, tag="g")
                    nc.tensor.matmul(psx[:], wx[:, 64 * g:64 * g + 64], seq_in,
                                     start=True, stop=True)
                    nc.scalar.activation(xp[:, g * nx:(g + 1) * nx], psx[:],
                                         AF.Copy)
                seq = pp.tile([64, nx + 2], F32, tag=f"seq{li}")
                nc.vector.memset(seq[:, 0:2], 0.0)
                for t in range(NEWT):
                    hprev = seq[:, 2 * t:2 * t + 2]
                    psg = pspool2.tile([64, 6], F32, tag="g")
                    for g in range(2):
                        nc.tensor.matmul(psg[:, 2 * g:2 * g + 2],
                                         wh[:, 64 * g:64 * g + 64], hprev,
                                         start=True, stop=False)
                        nc.tensor.matmul(
                            psg[:, 2 * g:2 * g + 2], ident[:],
                            xp[:, g * nx + 2 * t:g * nx + 2 * t + 2],
                            start=False, stop=True)
                    nc.tensor.matmul(psg[:, 4:6], wh[:, 128:192], hprev,
                                     start=True, stop=True)
                    zs = sp.tile([64, 2], F32, tag="zs")
                    rs = sp.tile([64, 2], F32, tag="rs")
                    rh = sp.tile([64, 2], F32, tag="rh")
                    hh = sp.tile([64, 2], F32, tag="hh")
                    nc.scalar.activation(zs[:], psg[:, 0:2], AF.Sigmoid,
                                         bias=gb[:, 4 * li:4 * li + 1])
                    nc.scalar.activation(rs[:], psg[:, 2:4], AF.Sigmoid,
                                         bias=gb[:, 4 * li + 1:4 * li + 2])
                    nc.scalar.activation(rh[:], psg[:, 4:6], AF.Identity,
                                         bias=gb[:, 4 * li + 2:4 * li + 3])
                    nc.vector.tensor_mul(rh[:], rs[:], rh[:])
                    nc.vector.tensor_add(rh[:], rh[:],
                                         xp[:, 2 * nx + 2 * t:2 * nx + 2 * t + 2])
                    nc.scalar.activation(hh[:], rh[:], AF.Tanh,
                                         bias=gb[:, 4 * li + 3:4 * li + 4])
                    hnew = seq[:, 2 * t + 2:2 * t + 4]
                    nc.vector.tensor_sub(hnew, hprev, hh[:])
                    nc.vector.tensor_mul(hnew, zs[:], hnew)
                    nc.vector.tensor_add(hnew, hnew, hh[:])
                seq_in = seq[:, 2:2 + nx]
                hT = seq[:, 2 * NEWT:2 * NEWT + 2]

            # ---------- heads -------------------------------------------------
            hbt = wt["headb"]
            psh = pspool2.tile([64, 2], F32, tag="g")
            d1o = sp.tile([64, 2], F32, tag="d1o")
            nc.tensor.matmul(psh[:], wt["d1w"][:], hT, start=True, stop=True)
            nc.scalar.activation(d1o[:], psh[:], AF.Relu, bias=hbt[:, 0:1])
            psh2 = pspool2.tile([32, 2], F32, tag="g")
            d2o = sp.tile([32, 2], F32, tag="d2o")
            nc.tensor.matmul(psh2[:], wt["d2w"][:], d1o[:], start=True, stop=True)
            nc.scalar.activation(d2o[:], psh2[:], AF.Relu, bias=hbt[0:32, 1:2])

            for nm, wn, width, bcol, out_d in [
                ("throw", "thrw", 64, 2, throw_d),
                ("tori", "toriw", 2, 3, tori_d),
            ]:
                psl = pspool2.tile([width, 2], F32, tag="g")
                logit = sp.tile([width, 2], F32, tag="lg" + nm)
                nc.tensor.matmul(psl[:], wt[wn][:], d2o[:], start=True, stop=True)
                nc.scalar.activation(logit[:], psl[:], AF.Identity,
                                     bias=hbt[0:width, bcol:bcol + 1])
                pst = pspool2.tile([2, width], F32, tag="g")
                nc.tensor.transpose(pst[:], logit[:], ident[0:width, 0:width])
                nmax = sp.tile([2, 1], F32, tag="nm" + nm)
                nc.vector.tensor_reduce(nmax[:], pst[:], op=mybir.AluOpType.max,
                                        axis=mybir.AxisListType.X, negate=True)
                ex = sp.tile([2, width], F32, tag="ex" + nm)
                ssum = sp.tile([2, 1], F32, tag="ss" + nm)
                nc.scalar.activation(ex[:], pst[:], AF.Exp, bias=nmax[:, 0:1],
                                     accum_out=ssum[:])
                rcp = sp.tile([2, 1], F32, tag="rc" + nm)
                nc.vector.reciprocal(rcp[:], ssum[:])
                sm = sp.tile([2, width], F32, tag="sm" + nm)
                nc.vector.tensor_scalar_mul(sm[:], ex[:], rcp[:, 0:1])
                nc.sync.dma_start(out_d[:], sm[:])

    nc.compile()
    return nc


_PROGRAM = None


def kernel(**inputs):
    global _PROGRAM
    if _PROGRAM is None:
        _PROGRAM = build_program()
    nc = _PROGRAM

    w = _pack_weights(inputs)
    items = _item_list()
    x = np.asarray(inputs["inputs"], np.float32)

    in_maps = []
    for c in range(N_CORES):
        xi = np.zeros((SLOTS, 8, 81, FR56), np.float32)
        for i in range(CNT[c]):
            b, t0 = items[OFFS[c] + i]
            xi[i] = _build_im2col(x[b, t0:t0 + WIN])
        m = {"x_im2col": xi}
        for nm, shape, dt in WEIGHT_SPECS:
            m[nm] = np.ascontiguousarray(w[nm].reshape(shape), np.float32)
        in_maps.append(m)

    res = bass_utils.run_bass_kernel_spmd(nc, in_maps,
                                          core_ids=list(range(N_CORES)))
    r0 = res.results[0]
    return np.asarray(r0["throw"], np.float32), np.asarray(r0["tori"], np.float32)


# revision 9
# speedup vs baseline: 1.3560x; 1.3560x over previous
"""Trainium2 Bass kernel for nn_CNN3DRNN (3D-CNN over sliding windows + stacked GRU).

Strategy:
  - 26 (batch, window) CNN items sharded over 8 cores (4 slots/core, zero-padded).
  - Conv3D as tap-packed f32r matmuls (fp32 PSUM accumulate): channel contraction
    on partitions over zero-padded activation planes; kd taps packed into the
    partition (K) dim via duplicated partition groups, kh/kw taps via AP offsets.
  - Deep software pipeline over frames (deepest-stage-first emission) so the
    evict->dup->matmul chains of consecutive layers overlap on PE/ACT/DVE/DMA.
  - Per-item GAP features AllGathered across cores; stacked GRU (wavefronted
    across its 3 layers) + heads run redundantly on every core; core 0 wins.
"""

import numpy as np

import concourse.bass as bass
import concourse.mybir as mybir
from concourse import bacc
from concourse import bass_utils
from concourse.tile import TileContext
from concourse.masks import make_identity

F32 = mybir.dt.float32
F32R = mybir.dt.float32r
AF = mybir.ActivationFunctionType


B, T, WIN, NEWT = 2, 20, 8, 13
N_ITEMS = B * NEWT  # 26
N_CORES = 8
SLOTS = 4
OFFS = [0, 4, 8, 11, 14, 17, 20, 23]
CNT = [4, 4, 3, 3, 3, 3, 3, 3]
GAP_N = float(WIN * 14 * 14)  # 1568

HP56, FR56 = 58, 58 * 58   # 3364
HP28, FR28 = 30, 30 * 30   # 900
HP14, FR14 = 16, 16 * 16   # 256

TAPS9 = [(kh, kw) for kh in range(3) for kw in range(3)]
TAPS27 = [(kd, kh, kw) for kd in range(3) for kh in range(3) for kw in range(3)]


def _item_list():
    return [(g // NEWT, g % NEWT) for g in range(N_ITEMS)]


def _build_im2col(win):
    """win: (8, 56, 56, 3) -> (8, 81, 3364) f32: rows (kd,kh,kw,c), padded."""
    xpad = np.zeros((10, 60, 60, 3), np.float32)
    xpad[1:9, 2:58, 2:58, :] = win
    out = np.empty((8, 81, 58, 58), np.float32)
    r = 0
    for kd, kh, kw in TAPS27:
        for c in range(3):
            out[:, r] = xpad[kd:kd + 8, kh:kh + 58, kw:kw + 58, c]
            r += 1
    return out.reshape(8, 81, FR56)


def _pack_weights(inp):
    w = {}
    w["wc11"] = np.asarray(inp["c11_k"], np.float32).reshape(81, 32)

    def dup3_pack(kk, cout):
        kk = np.asarray(kk, np.float32)
        cin = kk.shape[3]
        out = np.empty((3 * cin, 9 * cout), np.float32)
        for kh, kw in TAPS9:
            j = kh * 3 + kw
            out[:, j * cout:(j + 1) * cout] = kk[:, kh, kw].reshape(3 * cin, cout)
        return out

    def dup2_pack(kk, cout):
        kk = np.asarray(kk, np.float32)
        cin = kk.shape[3]
        wa = np.empty((2 * cin, 9 * cout), np.float32)
        wb = np.empty((cin, 9 * cout), np.float32)
        for kh, kw in TAPS9:
            j = kh * 3 + kw
            wa[:, j * cout:(j + 1) * cout] = kk[0:2, kh, kw].reshape(2 * cin, cout)
            wb[:, j * cout:(j + 1) * cout] = kk[2, kh, kw]
        return wa, wb

    w["wc12"] = dup3_pack(inp["c12_k"], 32)
    w["wc21"] = dup3_pack(inp["c21_k"], 64)
    w["wc22a"], w["wc22b"] = dup2_pack(inp["c22_k"], 64)
    w["wc31a"], w["wc31b"] = dup2_pack(inp["c31_k"], 128)

    k32 = np.asarray(inp["c32_k"], np.float32)
    wc32 = np.empty((128, 27 * 128), np.float32)
    for m, (kd, kh, kw) in enumerate(TAPS27):
        wc32[:, m * 128:(m + 1) * 128] = k32[kd, kh, kw]
    w["wc32"] = wc32

    cb = np.zeros((128, 6), np.float32)
    for i, nm in enumerate(["c11", "c12", "c21", "c22", "c31", "c32"]):
        b = np.asarray(inp[nm + "_b"], np.float32)
        cb[:len(b), i] = b
    w["convb"] = cb

    w["wg1x"] = np.asarray(inp["g1_wx"], np.float32) / GAP_N
    w["wg2x"] = np.asarray(inp["g2_wx"], np.float32)
    w["wg3x"] = np.asarray(inp["g3_wx"], np.float32)
    w["wg1h"] = np.asarray(inp["g1_wh"], np.float32)
    w["wg2h"] = np.asarray(inp["g2_wh"], np.float32)
    w["wg3h"] = np.asarray(inp["g3_wh"], np.float32)
    gb = np.zeros((64, 12), np.float32)
    for li, nm in enumerate(["g1", "g2", "g3"]):
        b = np.asarray(inp[nm + "_b"], np.float32)
        gb[:, li * 4 + 0] = b[0, 0:64] + b[1, 0:64]
        gb[:, li * 4 + 1] = b[0, 64:128] + b[1, 64:128]
        gb[:, li * 4 + 2] = b[1, 128:192]
        gb[:, li * 4 + 3] = b[0, 128:192]
    w["grub"] = gb

    w["d1w"] = np.asarray(inp["d1_w"], np.float32)
    w["d2w"] = np.asarray(inp["d2_w"], np.float32)
    w["thrw"] = np.asarray(inp["thr_w"], np.float32)
    w["toriw"] = np.asarray(inp["tori_w"], np.float32)
    hb = np.zeros((64, 4), np.float32)
    hb[0:64, 0] = np.asarray(inp["d1_b"], np.float32)
    hb[0:32, 1] = np.asarray(inp["d2_b"], np.float32)
    hb[0:64, 2] = np.asarray(inp["thr_b"], np.float32)
    hb[0:2, 3] = np.asarray(inp["tori_b"], np.float32)
    w["headb"] = hb
    return w


WEIGHT_SPECS = [
    ("wc11", [81, 32], F32R), ("wc12", [96, 9 * 32], F32R),
    ("wc21", [96, 9 * 64], F32R),
    ("wc22a", [128, 9 * 64], F32R), ("wc22b", [64, 9 * 64], F32R),
    ("wc31a", [128, 9 * 128], F32R), ("wc31b", [64, 9 * 128], F32R),
    ("wc32", [128, 27 * 128], F32R),
    ("convb", [128, 6], F32),
    ("wg1x", [128, 192], F32), ("wg2x", [64, 192], F32), ("wg3x", [64, 192], F32),
    ("wg1h", [64, 192], F32), ("wg2h", [64, 192], F32), ("wg3h", [64, 192], F32),
    ("grub", [64, 12], F32),
    ("d1w", [64, 64], F32), ("d2w", [64, 32], F32),
    ("thrw", [32, 64], F32), ("toriw", [32, 2], F32), ("headb", [64, 4], F32),
]


def _segments():
    segs = []
    for c in range(N_CORES):
        g0, n = OFFS[c], CNT[c]
        if g0 < NEWT < g0 + n:
            segs.append((c, 0, NEWT - g0, g0))
            segs.append((c, NEWT - g0, g0 + n - NEWT, NEWT))
        else:
            segs.append((c, 0, n, g0))
    return segs


def _dstcol(g):
    return (g % NEWT) * 2 + (g // NEWT)


def build_program():
    nc = bacc.Bacc()
    x_d = nc.dram_tensor("x_im2col", [SLOTS, 8, 81, FR56], F32R,
                         kind="ExternalInput")
    zeros_d = nc.dram_tensor("zeros", [128, FR56], F32R, kind="ExternalInput")
    wd = {}
    for nm, shape, dt in WEIGHT_SPECS:
        wd[nm] = nc.dram_tensor(nm, shape, dt, kind="ExternalInput")
    throw_d = nc.dram_tensor("throw", [2, 64], F32, kind="ExternalOutput")
    tori_d = nc.dram_tensor("tori", [2, 2], F32, kind="ExternalOutput")
    dbg_d = nc.dram_tensor("dbg_feats", [128, 2 * NEWT], F32, kind="ExternalOutput")
    feats_sh = nc.dram_tensor("feats_sh", [1, 128 * SLOTS], F32)
    feats_gd = nc.dram_tensor("feats_gd", [1, 128 * SLOTS * N_CORES], F32,
                              addr_space="Shared")

    with TileContext(nc) as tc:
        with (
            tc.tile_pool(name="wpool", bufs=1) as wpool,
            tc.tile_pool(name="persist", bufs=1) as pp,
            tc.tile_pool(name="small", bufs=3) as sp,
            tc.tile_pool(name="cnnps", bufs=4, space="PSUM") as pspool,
            tc.tile_pool(name="gru1ps", bufs=2, space="PSUM") as psg1pool,
            tc.tile_pool(name="gru2ps", bufs=2, space="PSUM") as psg2pool,
        ):
            wt = {}
            for nm, shape, dt in WEIGHT_SPECS:
                if nm in ("wc22b", "wc31b"):
                    # lhsT must share base_partition with its rhs (64)
                    full = wpool.tile([128, shape[1]], dt, tag=nm, name=nm)
                    nc.sync.dma_start(full[64:128, :], wd[nm][:])
                    wt[nm] = full
                else:
                    wt[nm] = wpool.tile(shape, dt, tag=nm, name=nm)
                    nc.sync.dma_start(wt[nm][:], wd[nm][:])
            ident = wpool.tile([64, 64], F32, tag="ident")
            make_identity(nc, ident[:])

            c11in = [pp.tile([81, FR56], F32R, tag=f"c11in{i}", name=f"c11in{i}")
                     for i in range(2)]
            c12ch = [pp.tile([96, FR56], F32R, tag=f"c12ch{i}", name=f"c12ch{i}")
                     for i in range(3)]
            c12out = [pp.tile([32, 3136], F32R, tag=f"c12o{i}", name=f"c12o{i}")
                      for i in range(2)]
            p1tmp = [pp.tile([32, 1568], F32R, tag=f"p1t{i}", name=f"p1t{i}")
                     for i in range(1)]
            c21ch = [pp.tile([96, FR28], F32R, tag=f"c21ch{i}", name=f"c21ch{i}")
                     for i in range(3)]
            c22ch = [pp.tile([128, FR28], F32R, tag=f"c22ch{i}", name=f"c22ch{i}")
                     for i in range(3)]
            c22out = [pp.tile([64, 784], F32R, tag=f"c22o{i}", name=f"c22o{i}")
                      for i in range(1)]
            p2tmp = [pp.tile([64, 392], F32R, tag=f"p2t{i}", name=f"p2t{i}")
                     for i in range(1)]
            c31in = pp.tile([128, 9 * FR14], F32R, tag="c31in")
            c32in = pp.tile([128, 10 * FR14], F32R, tag="c32in")
            evs = [pp.tile([128, 392], F32R, tag=f"evs{i}", name=f"evs{i}")
                   for i in range(1)]
            gapac = pp.tile([128, 4], F32, tag="gapac")
            feats = pp.tile([128, SLOTS], F32, tag="feats")

            def zfill(dst_ap):
                p, cols = dst_ap.shape[0], dst_ap.free_size()
                nc.sync.dma_start(dst_ap, zeros_d[0:p, 0:cols])

            for t_ in (c12ch + c12out + c21ch + c22ch + [c31in, c32in]):
                zfill(t_[:])

            cb = wt["convb"]

            def evict(idx, dst_ap, src_ap, bias_ap, accum=None):
                """Relu+bias eviction, alternating ACT / DVE by idx."""
                if accum is not None or idx % 2 == 0:
                    nc.scalar.activation(dst_ap, src_ap, AF.Relu, bias=bias_ap,
                                         accum_out=accum)
                else:
                    nc.vector.tensor_scalar(dst_ap, src_ap, bias_ap, 0.0,
                                            mybir.AluOpType.add,
                                            mybir.AluOpType.max,
                                            accum_out=accum)

            # ---------- per-stage emitters -----------------------------------
            def st_c11(item, f):
                xin = c11in[f % 2]
                nc.sync.dma_start(xin[:], x_d[item, f])
                ch = c12ch[f % 3]
                dst = ch[32:64, :].rearrange("k (h w) -> k h w", h=HP56)
                xv = xin[:].rearrange("k (h w) -> k h w", h=HP56)
                for tr in range(7):
                    ps = pspool.tile([128, 448], F32, tag="ps")
                    nc.tensor.matmul(
                        ps[0:32, :], wt["wc11"][:],
                        xv[:, 1 + tr * 8:9 + tr * 8, 1:57],
                        start=True, stop=True)
                    evict(tr, dst[:, 1 + tr * 8:9 + tr * 8, 1:57],
                          ps[0:32, :].rearrange("c (h w) -> c h w", h=8),
                          cb[0:32, 0:1])
                if f - 1 >= 0:
                    nc.sync.dma_start(ch[0:32, :], c12ch[(f - 1) % 3][32:64, :])
                    nc.sync.dma_start(c12ch[(f - 1) % 3][64:96, :], ch[32:64, :])

            def st_c12(item, d):
                chv = c12ch[d % 3][:].rearrange("k (h w) -> k h w", h=HP56)
                co = c12out[d % 2]
                for tr in range(7):
                    ps = pspool.tile([128, 448], F32, tag="ps")
                    for j, (kh, kw) in enumerate(TAPS9):
                        nc.tensor.matmul(
                            ps[0:32, :], wt["wc12"][:, j * 32:(j + 1) * 32],
                            chv[:, tr * 8 + kh: tr * 8 + kh + 8, kw:kw + 56],
                            start=(j == 0), stop=(j == 8))
                    evict(tr, co[:, tr * 448:(tr + 1) * 448], ps[0:32, :],
                          cb[0:32, 1:2])
                # pool1(d) -> c21 chunk group1 (frame d)
                pt = p1tmp[0]
                cov = co[:].rearrange("c (h wo wi) -> c h wo wi", h=56, wi=2)
                ptv = pt[:].rearrange("c (h w) -> c h w", h=56)
                nc.vector.tensor_max(ptv[:], cov[:, :, :, 0], cov[:, :, :, 1])
                ptv2 = pt[:].rearrange("c (hp hh w) -> c hp hh w", hp=28, hh=2)
                dstp = c21ch[d % 3][32:64, :].rearrange("c (h w) -> c h w", h=HP28)
                nc.vector.tensor_max(dstp[:, 1:29, 1:29],
                                     ptv2[:, :, 0, :], ptv2[:, :, 1, :])
                if d - 1 >= 0:
                    nc.sync.dma_start(c21ch[d % 3][0:32, :],
                                      c21ch[(d - 1) % 3][32:64, :])
                    nc.sync.dma_start(c21ch[(d - 1) % 3][64:96, :],
                                      c21ch[d % 3][32:64, :])

            def st_c21(item, d):
                chv = c21ch[d % 3][:].rearrange("k (h w) -> k h w", h=HP28)
                dst = c22ch[d % 3][64:128, :].rearrange("k (h w) -> k h w", h=HP28)
                for tr in range(2):
                    ps = pspool.tile([128, 392], F32, tag="ps")
                    for j, (kh, kw) in enumerate(TAPS9):
                        nc.tensor.matmul(
                            ps[0:64, :], wt["wc21"][:, j * 64:(j + 1) * 64],
                            chv[:, tr * 14 + kh: tr * 14 + kh + 14, kw:kw + 28],
                            start=(j == 0), stop=(j == 8))
                    evict(tr, dst[:, 1 + tr * 14:15 + tr * 14, 1:29],
                          ps[0:64, :].rearrange("c (h w) -> c h w", h=14),
                          cb[0:64, 2:3])
                if d + 1 <= 7:
                    nc.sync.dma_start(c22ch[(d + 1) % 3][0:64, :],
                                      c22ch[d % 3][64:128, :])

            def st_c22(item, d):
                v_a = c22ch[d % 3][:].rearrange("k (h w) -> k h w", h=HP28)
                v_b = c22ch[(d + 1) % 3][64:128, :].rearrange(
                    "k (h w) -> k h w", h=HP28)
                co2 = c22out[0]
                for tr in range(2):
                    ps = pspool.tile([128, 392], F32, tag="ps")
                    for j, (kh, kw) in enumerate(TAPS9):
                        nc.tensor.matmul(
                            ps[0:64, :], wt["wc22a"][:, j * 64:(j + 1) * 64],
                            v_a[:, tr * 14 + kh: tr * 14 + kh + 14, kw:kw + 28],
                            start=(j == 0), stop=False)
                    for j, (kh, kw) in enumerate(TAPS9):
                        nc.tensor.matmul(
                            ps[0:64, :], wt["wc22b"][64:128, j * 64:(j + 1) * 64],
                            v_b[:, tr * 14 + kh: tr * 14 + kh + 14, kw:kw + 28],
                            start=False, stop=(j == 8))
                    evict(tr, co2[:, tr * 392:(tr + 1) * 392], ps[0:64, :],
                          cb[0:64, 3:4])
                # pool2(d) -> c31in top half, slot d (frame d)
                pt2 = p2tmp[0]
                cov2 = co2[:].rearrange("c (h wo wi) -> c h wo wi", h=28, wi=2)
                pt2v = pt2[:].rearrange("c (h w) -> c h w", h=28)
                nc.vector.tensor_max(pt2v[:], cov2[:, :, :, 0], cov2[:, :, :, 1])
                pt2v2 = pt2[:].rearrange("c (hp hh w) -> c hp hh w", hp=14, hh=2)
                dst31 = c31in[64:128, :].rearrange("k (s h w) -> k s h w",
                                                   s=9, h=HP14)
                nc.vector.tensor_max(dst31[:, d, 1:15, 1:15],
                                     pt2v2[:, :, 0, :], pt2v2[:, :, 1, :])
                nc.sync.dma_start(c31in[0:64, (d + 1) * FR14:(d + 2) * FR14],
                                  c31in[64:128, d * FR14:(d + 1) * FR14])

            def st_c31(item, q):
                v31a = c31in[:].rearrange("k (s h w) -> k s h w", s=9, h=HP14)
                v31b = c31in[64:128, :].rearrange("k (s h w) -> k s h w",
                                                  s=9, h=HP14)
                ps = pspool.tile([128, 392], F32, tag="ps")
                for j, (kh, kw) in enumerate(TAPS9):
                    nc.tensor.matmul(
                        ps[:], wt["wc31a"][:, j * 128:(j + 1) * 128],
                        v31a[:, 2 * q:2 * q + 2, kh:kh + 14, kw:kw + 14],
                        start=(j == 0), stop=False)
                for j, (kh, kw) in enumerate(TAPS9):
                    nc.tensor.matmul(
                        ps[:], wt["wc31b"][64:128, j * 128:(j + 1) * 128],
                        v31b[:, 2 * q + 1:2 * q + 3, kh:kh + 14, kw:kw + 14],
                        start=False, stop=(j == 8))
                dst32 = c32in[:].rearrange("k (s h w) -> k s h w", s=10, h=HP14)
                evict(q, dst32[:, 2 * q + 1:2 * q + 3, 1:15, 1:15],
                      ps[:].rearrange("c (s h w) -> c s h w", s=2, h=14),
                      cb[:, 4:5])

            def st_c32(item, q):
                v32 = c32in[:].rearrange("k (s h w) -> k s h w", s=10, h=HP14)
                ps = pspool.tile([128, 392], F32, tag="ps")
                for m, (kd, kh, kw) in enumerate(TAPS27):
                    nc.tensor.matmul(
                        ps[:], wt["wc32"][:, m * 128:(m + 1) * 128],
                        v32[:, 2 * q + kd:2 * q + kd + 2, kh:kh + 14, kw:kw + 14],
                        start=(m == 0), stop=(m == 26))
                evict(q, evs[0][:], ps[:], cb[:, 5:6],
                      accum=gapac[:, q:q + 1])

            # ---------- CNN: deep pipeline, deepest stage first ----------------
            for item in range(SLOTS):
                for f in range(17):
                    if f == 12:
                        st_c32(item, 0)
                    if f == 14:
                        st_c32(item, 1)
                    if f == 16:
                        st_c32(item, 2)
                        st_c32(item, 3)
                    if f in (9, 11, 13, 15):
                        st_c31(item, (f - 9) // 2)
                    if 0 <= f - 6 <= 7:
                        st_c22(item, f - 6)
                    if 0 <= f - 4 <= 7:
                        st_c21(item, f - 4)
                    if 0 <= f - 2 <= 7:
                        st_c12(item, f - 2)
                    if f <= 7:
                        st_c11(item, f)
                    # boundary zero-fills (frame -1 / frame 8 surrogates)
                    if f == 0:
                        zfill(c12ch[0][0:32, :])
                    if f == 1:
                        zfill(c21ch[0][0:32, :])
                    if f == 3:
                        zfill(c22ch[0][0:64, :])
                    if f == 7:
                        zfill(c12ch[1][64:96, :])
                    if f == 9:
                        zfill(c21ch[1][64:96, :])
                    if f == 12:
                        zfill(c22ch[2][64:128, :])
                nc.vector.reduce_sum(feats[:, item:item + 1], gapac[:],
                                     axis=mybir.AxisListType.X)

            # ---------- gather features across cores --------------------------
            nc.sync.dma_start(
                feats_sh[:].rearrange("o (p i) -> (o p) i", p=128), feats[:])
            nc.gpsimd.collective_compute(
                "AllGather", mybir.AluOpType.bypass,
                ins=[feats_sh.ap().opt()], outs=[feats_gd.ap().opt()],
                replica_groups=[list(range(N_CORES))],
            )
            ford = pp.tile([128, 2 * NEWT], F32, tag="ford")
            gd3 = feats_gd.ap().rearrange("o (c p i) -> (o c) p i",
                                          c=N_CORES, p=128)
            for (c, i0, n, g0) in _segments():
                nc.sync.dma_start(
                    ford[:, _dstcol(g0):_dstcol(g0) + 2 * n - 1:2],
                    gd3[c, :, i0:i0 + n])

            nc.sync.dma_start(dbg_d[:], ford[:])

            # ---------- stacked GRU: wavefront across layers ------------------
            gb = wt["grub"]
            seqs = []
            for li in range(3):
                seq = pp.tile([64, 2 * NEWT + 2], F32, tag=f"seq{li}",
                              name=f"seq{li}")
                nc.vector.memset(seq[:, 0:2], 0.0)
                seqs.append(seq)

            def gru_step(li, t):
                wx = wt[["wg1x", "wg2x", "wg3x"][li]]
                wh = wt[["wg1h", "wg2h", "wg3h"][li]]
                seq = seqs[li]
                hprev = seq[:, 2 * t:2 * t + 2]
                x_in = (ford[:, 2 * t:2 * t + 2] if li == 0
                        else seqs[li - 1][:, 2 * t + 2:2 * t + 4])
                pzr = psg1pool.tile([64, 4], F32, tag="g1")
                phx = psg2pool.tile([64, 4], F32, tag="g2")
                for g in range(2):
                    nc.tensor.matmul(pzr[:, 2 * g:2 * g + 2],
                                     wh[:, 64 * g:64 * g + 64], hprev,
                                     start=True, stop=False)
                    nc.tensor.matmul(pzr[:, 2 * g:2 * g + 2],
                                     wx[:, 64 * g:64 * g + 64], x_in,
                                     start=False, stop=True)
                nc.tensor.matmul(phx[:, 0:2], wh[:, 128:192], hprev,
                                 start=True, stop=True)
                nc.tensor.matmul(phx[:, 2:4], wx[:, 128:192], x_in,
                                 start=True, stop=True)
                zs = sp.tile([64, 2], F32, tag="zs")
                rs = sp.tile([64, 2], F32, tag="rs")
                rh = sp.tile([64, 2], F32, tag="rh")
                hh = sp.tile([64, 2], F32, tag="hh")
                nc.scalar.activation(zs[:], pzr[:, 0:2], AF.Sigmoid,
                                     bias=gb[:, 4 * li:4 * li + 1])
                nc.scalar.activation(rs[:], pzr[:, 2:4], AF.Sigmoid,
                                     bias=gb[:, 4 * li + 1:4 * li + 2])
                # rh = (rec_h + b1_h) * r
                nc.vector.scalar_tensor_tensor(
                    rh[:], phx[:, 0:2], gb[:, 4 * li + 2:4 * li + 3], rs[:],
                    mybir.AluOpType.add, mybir.AluOpType.mult)
                nc.vector.tensor_add(rh[:], rh[:], phx[:, 2:4])
                nc.scalar.activation(hh[:], rh[:], AF.Tanh,
                                     bias=gb[:, 4 * li + 3:4 * li + 4])
                hnew = seq[:, 2 * t + 2:2 * t + 4]
                nc.vector.tensor_sub(hnew, hprev, hh[:])
                nc.vector.tensor_mul(hnew, zs[:], hnew)
                nc.vector.tensor_add(hnew, hnew, hh[:])

            for rnd in range(NEWT + 2):
                for li in range(3):
                    t = rnd - li
                    if 0 <= t < NEWT:
                        gru_step(li, t)
            hT = seqs[2][:, 2 * NEWT:2 * NEWT + 2]

            # ---------- heads -------------------------------------------------
            hbt = wt["headb"]
            psh = psg1pool.tile([64, 2], F32, tag="g1")
            d1o = sp.tile([64, 2], F32, tag="d1o")
            nc.tensor.matmul(psh[:], wt["d1w"][:], hT, start=True, stop=True)
            nc.scalar.activation(d1o[:], psh[:], AF.Relu, bias=hbt[:, 0:1])
            psh2 = psg1pool.tile([32, 2], F32, tag="g1")
            d2o = sp.tile([32, 2], F32, tag="d2o")
            nc.tensor.matmul(psh2[:], wt["d2w"][:], d1o[:], start=True, stop=True)
            nc.scalar.activation(d2o[:], psh2[:], AF.Relu, bias=hbt[0:32, 1:2])

            for nm, wn, width, bcol, out_d in [
                ("throw", "thrw", 64, 2, throw_d),
                ("tori", "toriw", 2, 3, tori_d),
            ]:
                psl = psg1pool.tile([width, 2], F32, tag="g1")
                logit = sp.tile([width, 2], F32, tag="lg" + nm)
                nc.tensor.matmul(psl[:], wt[wn][:], d2o[:], start=True, stop=True)
                nc.scalar.activation(logit[:], psl[:], AF.Identity,
                                     bias=hbt[0:width, bcol:bcol + 1])
                pst = psg2pool.tile([2, width], F32, tag="g2")
                nc.tensor.transpose(pst[:], logit[:], ident[0:width, 0:width])
                nmax = sp.tile([2, 1], F32, tag="nm" + nm)
                nc.vector.tensor_reduce(nmax[:], pst[:], op=mybir.AluOpType.max,
                                        axis=mybir.AxisListType.X, negate=True)
                ex = sp.tile([2, width], F32, tag="ex" + nm)
                ssum = sp.tile([2, 1], F32, tag="ss" + nm)
                nc.scalar.activation(ex[:], pst[:], AF.Exp, bias=nmax[:, 0:1],
                                     accum_out=ssum[:])
                rcp = sp.tile([2, 1], F32, tag="rc" + nm)
                nc.vector.reciprocal(rcp[:], ssum[:])
                sm = sp.tile([2, width], F32, tag="sm" + nm)
                nc.vector.tensor_scalar_mul(sm[:], ex[:], rcp[:, 0:1])
                nc.sync.dma_start(out_d[:], sm[:])

    nc.compile()
    return nc


_PROGRAM = None


def kernel(**inputs):
    global _PROGRAM
    if _PROGRAM is None:
        _PROGRAM = build_program()
    nc = _PROGRAM

    w = _pack_weights(inputs)
    items = _item_list()
    x = np.asarray(inputs["inputs"], np.float32)

    in_maps = []
    for c in range(N_CORES):
        xi = np.zeros((SLOTS, 8, 81, FR56), np.float32)
        for i in range(CNT[c]):
            b, t0 = items[OFFS[c] + i]
            xi[i] = _build_im2col(x[b, t0:t0 + WIN])
        m = {"x_im2col": xi, "zeros": np.zeros((128, FR56), np.float32)}
        for nm, shape, dt in WEIGHT_SPECS:
            m[nm] = np.ascontiguousarray(w[nm].reshape(shape), np.float32)
        in_maps.append(m)

    res = bass_utils.run_bass_kernel_spmd(nc, in_maps,
                                          core_ids=list(range(N_CORES)))
    r0 = res.results[0]
    kernel.last_debug = {k: np.asarray(v) for k, v in r0.items()}
    return np.asarray(r0["throw"], np.float32), np.asarray(r0["tori"], np.float32)


# revision 10
# speedup vs baseline: 1.3594x; 1.0025x over previous
"""Trainium2 Bass kernel for nn_CNN3DRNN (3D-CNN over sliding windows + stacked GRU).

Strategy:
  - 26 (batch, window) CNN items sharded over 8 cores (4 slots/core, zero-padded).
  - Conv3D as tap-packed bf16 matmuls (fp32 PSUM accumulate): channel contraction
    on partitions over zero-padded activation planes; kd taps packed into the
    partition (K) dim via duplicated partition groups, kh/kw taps via AP offsets.
  - Deep software pipeline over frames (deepest-stage-first emission) so the
    evict->dup->matmul chains of consecutive layers overlap on PE/ACT/DVE/DMA.
  - Per-item GAP features AllGathered across cores; stacked GRU (wavefronted
    across its 3 layers) + heads run redundantly on every core; core 0 wins.
"""

import numpy as np

import concourse.bass as bass
import concourse.mybir as mybir
from concourse import bacc
from concourse import bass_utils
from concourse.tile import TileContext
from concourse.masks import make_identity

F32 = mybir.dt.float32
import ml_dtypes
BF16 = mybir.dt.bfloat16
BF16_NP = ml_dtypes.bfloat16
AF = mybir.ActivationFunctionType


B, T, WIN, NEWT = 2, 20, 8, 13
N_ITEMS = B * NEWT  # 26
N_CORES = 8
SLOTS = 4
OFFS = [0, 4, 8, 11, 14, 17, 20, 23]
CNT = [4, 4, 3, 3, 3, 3, 3, 3]
GAP_N = float(WIN * 14 * 14)  # 1568

HP56, FR56 = 58, 58 * 58   # 3364
HP28, FR28 = 30, 30 * 30   # 900
HP14, FR14 = 16, 16 * 16   # 256

TAPS9 = [(kh, kw) for kh in range(3) for kw in range(3)]
TAPS27 = [(kd, kh, kw) for kd in range(3) for kh in range(3) for kw in range(3)]


def _item_list():
    return [(g // NEWT, g % NEWT) for g in range(N_ITEMS)]


def _build_im2col(win):
    """win: (8, 56, 56, 3) -> (8, 81, 3364) f32: rows (kd,kh,kw,c), padded."""
    xpad = np.zeros((10, 60, 60, 3), np.float32)
    xpad[1:9, 2:58, 2:58, :] = win
    xpad = xpad.astype(BF16_NP)
    out = np.empty((8, 81, 58, 58), BF16_NP)
    r = 0
    for kd, kh, kw in TAPS27:
        for c in range(3):
            out[:, r] = xpad[kd:kd + 8, kh:kh + 58, kw:kw + 58, c]
            r += 1
    return out.reshape(8, 81, FR56)


def _pack_weights(inp):
    w = {}
    w["wc11"] = np.asarray(inp["c11_k"], np.float32).reshape(81, 32)

    def dup3_pack(kk, cout):
        kk = np.asarray(kk, np.float32)
        cin = kk.shape[3]
        out = np.empty((3 * cin, 9 * cout), np.float32)
        for kh, kw in TAPS9:
            j = kh * 3 + kw
            out[:, j * cout:(j + 1) * cout] = kk[:, kh, kw].reshape(3 * cin, cout)
        return out

    def dup2_pack(kk, cout):
        kk = np.asarray(kk, np.float32)
        cin = kk.shape[3]
        wa = np.empty((2 * cin, 9 * cout), np.float32)
        wb = np.empty((cin, 9 * cout), np.float32)
        for kh, kw in TAPS9:
            j = kh * 3 + kw
            wa[:, j * cout:(j + 1) * cout] = kk[0:2, kh, kw].reshape(2 * cin, cout)
            wb[:, j * cout:(j + 1) * cout] = kk[2, kh, kw]
        return wa, wb

    w["wc12"] = dup3_pack(inp["c12_k"], 32)
    w["wc21"] = dup3_pack(inp["c21_k"], 64)
    w["wc22a"], w["wc22b"] = dup2_pack(inp["c22_k"], 64)
    w["wc31a"], w["wc31b"] = dup2_pack(inp["c31_k"], 128)

    k32 = np.asarray(inp["c32_k"], np.float32)
    wc32 = np.empty((128, 27 * 128), np.float32)
    for m, (kd, kh, kw) in enumerate(TAPS27):
        wc32[:, m * 128:(m + 1) * 128] = k32[kd, kh, kw]
    w["wc32"] = wc32

    cb = np.zeros((128, 6), np.float32)
    for i, nm in enumerate(["c11", "c12", "c21", "c22", "c31", "c32"]):
        b = np.asarray(inp[nm + "_b"], np.float32)
        cb[:len(b), i] = b
    w["convb"] = cb

    w["wg1x"] = np.asarray(inp["g1_wx"], np.float32) / GAP_N
    w["wg2x"] = np.asarray(inp["g2_wx"], np.float32)
    w["wg3x"] = np.asarray(inp["g3_wx"], np.float32)
    w["wg1h"] = np.asarray(inp["g1_wh"], np.float32)
    w["wg2h"] = np.asarray(inp["g2_wh"], np.float32)
    w["wg3h"] = np.asarray(inp["g3_wh"], np.float32)
    gb = np.zeros((64, 12), np.float32)
    for li, nm in enumerate(["g1", "g2", "g3"]):
        b = np.asarray(inp[nm + "_b"], np.float32)
        gb[:, li * 4 + 0] = b[0, 0:64] + b[1, 0:64]
        gb[:, li * 4 + 1] = b[0, 64:128] + b[1, 64:128]
        gb[:, li * 4 + 2] = b[1, 128:192]
        gb[:, li * 4 + 3] = b[0, 128:192]
    w["grub"] = gb

    w["d1w"] = np.asarray(inp["d1_w"], np.float32)
    w["d2w"] = np.asarray(inp["d2_w"], np.float32)
    w["thrw"] = np.asarray(inp["thr_w"], np.float32)
    w["toriw"] = np.asarray(inp["tori_w"], np.float32)
    hb = np.zeros((64, 4), np.float32)
    hb[0:64, 0] = np.asarray(inp["d1_b"], np.float32)
    hb[0:32, 1] = np.asarray(inp["d2_b"], np.float32)
    hb[0:64, 2] = np.asarray(inp["thr_b"], np.float32)
    hb[0:2, 3] = np.asarray(inp["tori_b"], np.float32)
    w["headb"] = hb
    return w


WEIGHT_SPECS = [
    ("wc11", [81, 32], BF16), ("wc12", [96, 9 * 32], BF16),
    ("wc21", [96, 9 * 64], BF16),
    ("wc22a", [128, 9 * 64], BF16), ("wc22b", [64, 9 * 64], BF16),
    ("wc31a", [128, 9 * 128], BF16), ("wc31b", [64, 9 * 128], BF16),
    ("wc32", [128, 27 * 128], BF16),
    ("convb", [128, 6], F32),
    ("wg1x", [128, 192], F32), ("wg2x", [64, 192], F32), ("wg3x", [64, 192], F32),
    ("wg1h", [64, 192], F32), ("wg2h", [64, 192], F32), ("wg3h", [64, 192], F32),
    ("grub", [64, 12], F32),
    ("d1w", [64, 64], F32), ("d2w", [64, 32], F32),
    ("thrw", [32, 64], F32), ("toriw", [32, 2], F32), ("headb", [64, 4], F32),
]


def _segments():
    segs = []
    for c in range(N_CORES):
        g0, n = OFFS[c], CNT[c]
        if g0 < NEWT < g0 + n:
            segs.append((c, 0, NEWT - g0, g0))
            segs.append((c, NEWT - g0, g0 + n - NEWT, NEWT))
        else:
            segs.append((c, 0, n, g0))
    return segs


def _dstcol(g):
    return (g % NEWT) * 2 + (g // NEWT)


def build_program():
    nc = bacc.Bacc()
    x_d = nc.dram_tensor("x_im2col", [SLOTS, 8, 81, FR56], BF16,
                         kind="ExternalInput")
    zeros_d = nc.dram_tensor("zeros", [128, FR56], BF16, kind="ExternalInput")
    wd = {}
    for nm, shape, dt in WEIGHT_SPECS:
        wd[nm] = nc.dram_tensor(nm, shape, dt, kind="ExternalInput")
    throw_d = nc.dram_tensor("throw", [2, 64], F32, kind="ExternalOutput")
    tori_d = nc.dram_tensor("tori", [2, 2], F32, kind="ExternalOutput")
    dbg_d = nc.dram_tensor("dbg_feats", [128, 2 * NEWT], F32, kind="ExternalOutput")
    feats_sh = nc.dram_tensor("feats_sh", [1, 128 * SLOTS], F32)
    feats_gd = nc.dram_tensor("feats_gd", [1, 128 * SLOTS * N_CORES], F32,
                              addr_space="Shared")

    with TileContext(nc) as tc:
        with (
            tc.tile_pool(name="wpool", bufs=1) as wpool,
            tc.tile_pool(name="persist", bufs=1) as pp,
            tc.tile_pool(name="small", bufs=3) as sp,
            tc.tile_pool(name="cnnps", bufs=4, space="PSUM") as pspool,
            tc.tile_pool(name="gru1ps", bufs=2, space="PSUM") as psg1pool,
            tc.tile_pool(name="gru2ps", bufs=2, space="PSUM") as psg2pool,
        ):
            wt = {}
            for nm, shape, dt in WEIGHT_SPECS:
                if nm in ("wc22b", "wc31b"):
                    # lhsT must share base_partition with its rhs (64)
                    full = wpool.tile([128, shape[1]], dt, tag=nm, name=nm)
                    nc.sync.dma_start(full[64:128, :], wd[nm][:])
                    wt[nm] = full
                else:
                    wt[nm] = wpool.tile(shape, dt, tag=nm, name=nm)
                    nc.sync.dma_start(wt[nm][:], wd[nm][:])
            ident = wpool.tile([64, 64], F32, tag="ident")
            make_identity(nc, ident[:])

            c11in = [pp.tile([81, FR56], BF16, tag=f"c11in{i}", name=f"c11in{i}")
                     for i in range(2)]
            c12ch = [pp.tile([96, FR56], BF16, tag=f"c12ch{i}", name=f"c12ch{i}")
                     for i in range(3)]
            c12out = [pp.tile([32, 3136], BF16, tag=f"c12o{i}", name=f"c12o{i}")
                      for i in range(2)]
            p1tmp = [pp.tile([32, 1568], BF16, tag=f"p1t{i}", name=f"p1t{i}")
                     for i in range(1)]
            c21ch = [pp.tile([96, FR28], BF16, tag=f"c21ch{i}", name=f"c21ch{i}")
                     for i in range(3)]
            c22ch = [pp.tile([128, FR28], BF16, tag=f"c22ch{i}", name=f"c22ch{i}")
                     for i in range(3)]
            c22out = [pp.tile([64, 784], BF16, tag=f"c22o{i}", name=f"c22o{i}")
                      for i in range(1)]
            p2tmp = [pp.tile([64, 392], BF16, tag=f"p2t{i}", name=f"p2t{i}")
                     for i in range(1)]
            c31in = pp.tile([128, 9 * FR14], BF16, tag="c31in")
            c32in = pp.tile([128, 10 * FR14], BF16, tag="c32in")
            evs = [pp.tile([128, 392], BF16, tag=f"evs{i}", name=f"evs{i}")
                   for i in range(1)]
            gapac = pp.tile([128, 4], F32, tag="gapac")
            feats = pp.tile([128, SLOTS], F32, tag="feats")

            def zfill(dst_ap):
                p, cols = dst_ap.shape[0], dst_ap.free_size()
                nc.sync.dma_start(dst_ap, zeros_d[0:p, 0:cols])

            for t_ in (c12ch + c12out + c21ch + c22ch + [c31in, c32in]):
                zfill(t_[:])

            cb = wt["convb"]

            def evict(idx, dst_ap, src_ap, bias_ap, accum=None):
                """Relu+bias eviction, alternating ACT / DVE by idx."""
                if accum is not None or idx % 2 == 0:
                    nc.scalar.activation(dst_ap, src_ap, AF.Relu, bias=bias_ap,
                                         accum_out=accum)
                else:
                    nc.vector.tensor_scalar(dst_ap, src_ap, bias_ap, 0.0,
                                            mybir.AluOpType.add,
                                            mybir.AluOpType.max,
                                            accum_out=accum)

            # ---------- per-stage emitters -----------------------------------
            def st_c11(item, f):
                xin = c11in[f % 2]
                nc.sync.dma_start(xin[:], x_d[item, f])
                ch = c12ch[f % 3]
                dst = ch[32:64, :].rearrange("k (h w) -> k h w", h=HP56)
                xv = xin[:].rearrange("k (h w) -> k h w", h=HP56)
                for tr in range(7):
                    ps = pspool.tile([128, 448], F32, tag="ps")
                    nc.tensor.matmul(
                        ps[0:32, :], wt["wc11"][:],
                        xv[:, 1 + tr * 8:9 + tr * 8, 1:57],
                        start=True, stop=True)
                    evict(tr, dst[:, 1 + tr * 8:9 + tr * 8, 1:57],
                          ps[0:32, :].rearrange("c (h w) -> c h w", h=8),
                          cb[0:32, 0:1])
                if f - 1 >= 0:
                    nc.sync.dma_start(ch[0:32, :], c12ch[(f - 1) % 3][32:64, :])
                    nc.sync.dma_start(c12ch[(f - 1) % 3][64:96, :], ch[32:64, :])

            def st_c12(item, d):
                chv = c12ch[d % 3][:].rearrange("k (h w) -> k h w", h=HP56)
                co = c12out[d % 2]
                for tr in range(7):
                    ps = pspool.tile([128, 448], F32, tag="ps")
                    for j, (kh, kw) in enumerate(TAPS9):
                        nc.tensor.matmul(
                            ps[0:32, :], wt["wc12"][:, j * 32:(j + 1) * 32],
                            chv[:, tr * 8 + kh: tr * 8 + kh + 8, kw:kw + 56],
                            start=(j == 0), stop=(j == 8))
                    evict(tr, co[:, tr * 448:(tr + 1) * 448], ps[0:32, :],
                          cb[0:32, 1:2])
                # pool1(d) -> c21 chunk group1 (frame d)
                pt = p1tmp[0]
                cov = co[:].rearrange("c (h wo wi) -> c h wo wi", h=56, wi=2)
                ptv = pt[:].rearrange("c (h w) -> c h w", h=56)
                nc.vector.tensor_max(ptv[:], cov[:, :, :, 0], cov[:, :, :, 1])
                ptv2 = pt[:].rearrange("c (hp hh w) -> c hp hh w", hp=28, hh=2)
                dstp = c21ch[d % 3][32:64, :].rearrange("c (h w) -> c h w", h=HP28)
                nc.vector.tensor_max(dstp[:, 1:29, 1:29],
                                     ptv2[:, :, 0, :], ptv2[:, :, 1, :])
                if d - 1 >= 0:
                    nc.sync.dma_start(c21ch[d % 3][0:32, :],
                                      c21ch[(d - 1) % 3][32:64, :])
                    nc.sync.dma_start(c21ch[(d - 1) % 3][64:96, :],
                                      c21ch[d % 3][32:64, :])

            def st_c21(item, d):
                chv = c21ch[d % 3][:].rearrange("k (h w) -> k h w", h=HP28)
                dst = c22ch[d % 3][64:128, :].rearrange("k (h w) -> k h w", h=HP28)
                for tr in range(2):
                    ps = pspool.tile([128, 392], F32, tag="ps")
                    for j, (kh, kw) in enumerate(TAPS9):
                        nc.tensor.matmul(
                            ps[0:64, :], wt["wc21"][:, j * 64:(j + 1) * 64],
                            chv[:, tr * 14 + kh: tr * 14 + kh + 14, kw:kw + 28],
                            start=(j == 0), stop=(j == 8))
                    evict(tr, dst[:, 1 + tr * 14:15 + tr * 14, 1:29],
                          ps[0:64, :].rearrange("c (h w) -> c h w", h=14),
                          cb[0:64, 2:3])
                if d + 1 <= 7:
                    nc.sync.dma_start(c22ch[(d + 1) % 3][0:64, :],
                                      c22ch[d % 3][64:128, :])

            def st_c22(item, d):
                v_a = c22ch[d % 3][:].rearrange("k (h w) -> k h w", h=HP28)
                v_b = c22ch[(d + 1) % 3][64:128, :].rearrange(
                    "k (h w) -> k h w", h=HP28)
                co2 = c22out[0]
                for tr in range(2):
                    ps = pspool.tile([128, 392], F32, tag="ps")
                    for j, (kh, kw) in enumerate(TAPS9):
                        nc.tensor.matmul(
                            ps[0:64, :], wt["wc22a"][:, j * 64:(j + 1) * 64],
                            v_a[:, tr * 14 + kh: tr * 14 + kh + 14, kw:kw + 28],
                            start=(j == 0), stop=False)
                    for j, (kh, kw) in enumerate(TAPS9):
                        nc.tensor.matmul(
                            ps[0:64, :], wt["wc22b"][64:128, j * 64:(j + 1) * 64],
                            v_b[:, tr * 14 + kh: tr * 14 + kh + 14, kw:kw + 28],
                            start=False, stop=(j == 8))
                    evict(tr, co2[:, tr * 392:(tr + 1) * 392], ps[0:64, :],
                          cb[0:64, 3:4])
                # pool2(d) -> c31in top half, slot d (frame d)
                pt2 = p2tmp[0]
                cov2 = co2[:].rearrange("c (h wo wi) -> c h wo wi", h=28, wi=2)
                pt2v = pt2[:].rearrange("c (h w) -> c h w", h=28)
                nc.vector.tensor_max(pt2v[:], cov2[:, :, :, 0], cov2[:, :, :, 1])
                pt2v2 = pt2[:].rearrange("c (hp hh w) -> c hp hh w", hp=14, hh=2)
                dst31 = c31in[64:128, :].rearrange("k (s h w) -> k s h w",
                                                   s=9, h=HP14)
                nc.vector.tensor_max(dst31[:, d, 1:15, 1:15],
                                     pt2v2[:, :, 0, :], pt2v2[:, :, 1, :])
                nc.sync.dma_start(c31in[0:64, (d + 1) * FR14:(d + 2) * FR14],
                                  c31in[64:128, d * FR14:(d + 1) * FR14])

            def st_c31(item, q):
                v31a = c31in[:].rearrange("k (s h w) -> k s h w", s=9, h=HP14)
                v31b = c31in[64:128, :].rearrange("k (s h w) -> k s h w",
                                                  s=9, h=HP14)
                ps = pspool.tile([128, 392], F32, tag="ps")
                for j, (kh, kw) in enumerate(TAPS9):
                    nc.tensor.matmul(
                        ps[:], wt["wc31a"][:, j * 128:(j + 1) * 128],
                        v31a[:, 2 * q:2 * q + 2, kh:kh + 14, kw:kw + 14],
                        start=(j == 0), stop=False)
                for j, (kh, kw) in enumerate(TAPS9):
                    nc.tensor.matmul(
                        ps[:], wt["wc31b"][64:128, j * 128:(j + 1) * 128],
                        v31b[:, 2 * q + 1:2 * q + 3, kh:kh + 14, kw:kw + 14],
                        start=False, stop=(j == 8))
                dst32 = c32in[:].rearrange("k (s h w) -> k s h w", s=10, h=HP14)
                evict(q, dst32[:, 2 * q + 1:2 * q + 3, 1:15, 1:15],
                      ps[:].rearrange("c (s h w) -> c s h w", s=2, h=14),
                      cb[:, 4:5])

            def st_c32(item, q):
                v32 = c32in[:].rearrange("k (s h w) -> k s h w", s=10, h=HP14)
                ps = pspool.tile([128, 392], F32, tag="ps")
                for m, (kd, kh, kw) in enumerate(TAPS27):
                    nc.tensor.matmul(
                        ps[:], wt["wc32"][:, m * 128:(m + 1) * 128],
                        v32[:, 2 * q + kd:2 * q + kd + 2, kh:kh + 14, kw:kw + 14],
                        start=(m == 0), stop=(m == 26))
                evict(q, evs[0][:], ps[:], cb[:, 5:6],
                      accum=gapac[:, q:q + 1])

            # ---------- CNN: deep pipeline, deepest stage first ----------------
            for item in range(SLOTS):
                for f in range(17):
                    if f == 12:
                        st_c32(item, 0)
                    if f == 14:
                        st_c32(item, 1)
                    if f == 16:
                        st_c32(item, 2)
                        st_c32(item, 3)
                    if f in (9, 11, 13, 15):
                        st_c31(item, (f - 9) // 2)
                    if 0 <= f - 6 <= 7:
                        st_c22(item, f - 6)
                    if 0 <= f - 4 <= 7:
                        st_c21(item, f - 4)
                    if 0 <= f - 2 <= 7:
                        st_c12(item, f - 2)
                    if f <= 7:
                        st_c11(item, f)
                    # boundary zero-fills (frame -1 / frame 8 surrogates)
                    if f == 0:
                        zfill(c12ch[0][0:32, :])
                    if f == 1:
                        zfill(c21ch[0][0:32, :])
                    if f == 3:
                        zfill(c22ch[0][0:64, :])
                    if f == 7:
                        zfill(c12ch[1][64:96, :])
                    if f == 9:
                        zfill(c21ch[1][64:96, :])
                    if f == 12:
                        zfill(c22ch[2][64:128, :])
                nc.vector.reduce_sum(feats[:, item:item + 1], gapac[:],
                                     axis=mybir.AxisListType.X)

            # ---------- gather features across cores --------------------------
            nc.sync.dma_start(
                feats_sh[:].rearrange("o (p i) -> (o p) i", p=128), feats[:])
            nc.gpsimd.collective_compute(
                "AllGather", mybir.AluOpType.bypass,
                ins=[feats_sh.ap().opt()], outs=[feats_gd.ap().opt()],
                replica_groups=[list(range(N_CORES))],
            )
            ford = pp.tile([128, 2 * NEWT], F32, tag="ford")
            gd3 = feats_gd.ap().rearrange("o (c p i) -> (o c) p i",
                                          c=N_CORES, p=128)
            for (c, i0, n, g0) in _segments():
                nc.sync.dma_start(
                    ford[:, _dstcol(g0):_dstcol(g0) + 2 * n - 1:2],
                    gd3[c, :, i0:i0 + n])

            nc.sync.dma_start(dbg_d[:], ford[:])

            # ---------- stacked GRU: wavefront across layers ------------------
            gb = wt["grub"]
            seqs = []
            for li in range(3):
                seq = pp.tile([64, 2 * NEWT + 2], F32, tag=f"seq{li}",
                              name=f"seq{li}")
                nc.vector.memset(seq[:, 0:2], 0.0)
                seqs.append(seq)

            def gru_step(li, t):
                wx = wt[["wg1x", "wg2x", "wg3x"][li]]
                wh = wt[["wg1h", "wg2h", "wg3h"][li]]
                seq = seqs[li]
                hprev = seq[:, 2 * t:2 * t + 2]
                x_in = (ford[:, 2 * t:2 * t + 2] if li == 0
                        else seqs[li - 1][:, 2 * t + 2:2 * t + 4])
                pzr = psg1pool.tile([64, 4], F32, tag="g1")
                phx = psg2pool.tile([64, 4], F32, tag="g2")
                for g in range(2):
                    nc.tensor.matmul(pzr[:, 2 * g:2 * g + 2],
                                     wh[:, 64 * g:64 * g + 64], hprev,
                                     start=True, stop=False)
                    nc.tensor.matmul(pzr[:, 2 * g:2 * g + 2],
                                     wx[:, 64 * g:64 * g + 64], x_in,
                                     start=False, stop=True)
                nc.tensor.matmul(phx[:, 0:2], wh[:, 128:192], hprev,
                                 start=True, stop=True)
                nc.tensor.matmul(phx[:, 2:4], wx[:, 128:192], x_in,
                                 start=True, stop=True)
                zs = sp.tile([64, 2], F32, tag="zs")
                rs = sp.tile([64, 2], F32, tag="rs")
                rh = sp.tile([64, 2], F32, tag="rh")
                hh = sp.tile([64, 2], F32, tag="hh")
                nc.scalar.activation(zs[:], pzr[:, 0:2], AF.Sigmoid,
                                     bias=gb[:, 4 * li:4 * li + 1])
                nc.scalar.activation(rs[:], pzr[:, 2:4], AF.Sigmoid,
                                     bias=gb[:, 4 * li + 1:4 * li + 2])
                # rh = (rec_h + b1_h) * r
                nc.vector.scalar_tensor_tensor(
                    rh[:], phx[:, 0:2], gb[:, 4 * li + 2:4 * li + 3], rs[:],
                    mybir.AluOpType.add, mybir.AluOpType.mult)
                nc.vector.tensor_add(rh[:], rh[:], phx[:, 2:4])
                nc.scalar.activation(hh[:], rh[:], AF.Tanh,
                                     bias=gb[:, 4 * li + 3:4 * li + 4])
                hnew = seq[:, 2 * t + 2:2 * t + 4]
                nc.vector.tensor_sub(hnew, hprev, hh[:])
                nc.vector.tensor_mul(hnew, zs[:], hnew)
                nc.vector.tensor_add(hnew, hnew, hh[:])

            for rnd in range(NEWT + 2):
                for li in range(3):
                    t = rnd - li
                    if 0 <= t < NEWT:
                        gru_step(li, t)
            hT = seqs[2][:, 2 * NEWT:2 * NEWT + 2]

            # ---------- heads -------------------------------------------------
            hbt = wt["headb"]
            psh = psg1pool.tile([64, 2], F32, tag="g1")
            d1o = sp.tile([64, 2], F32, tag="d1o")
            nc.tensor.matmul(psh[:], wt["d1w"][:], hT, start=True, stop=True)
            nc.scalar.activation(d1o[:], psh[:], AF.Relu, bias=hbt[:, 0:1])
            psh2 = psg1pool.tile([32, 2], F32, tag="g1")
            d2o = sp.tile([32, 2], F32, tag="d2o")
            nc.tensor.matmul(psh2[:], wt["d2w"][:], d1o[:], start=True, stop=True)
            nc.scalar.activation(d2o[:], psh2[:], AF.Relu, bias=hbt[0:32, 1:2])

            for nm, wn, width, bcol, out_d in [
                ("throw", "thrw", 64, 2, throw_d),
                ("tori", "toriw", 2, 3, tori_d),
            ]:
                psl = psg1pool.tile([width, 2], F32, tag="g1")
                logit = sp.tile([width, 2], F32, tag="lg" + nm)
                nc.tensor.matmul(psl[:], wt[wn][:], d2o[:], start=True, stop=True)
                nc.scalar.activation(logit[:], psl[:], AF.Identity,
                                     bias=hbt[0:width, bcol:bcol + 1])
                pst = psg2pool.tile([2, width], F32, tag="g2")
                nc.tensor.transpose(pst[:], logit[:], ident[0:width, 0:width])
                nmax = sp.tile([2, 1], F32, tag="nm" + nm)
                nc.vector.tensor_reduce(nmax[:], pst[:], op=mybir.AluOpType.max,
                                        axis=mybir.AxisListType.X, negate=True)
                ex = sp.tile([2, width], F32, tag="ex" + nm)
                ssum = sp.tile([2, 1], F32, tag="ss" + nm)
                nc.scalar.activation(ex[:], pst[:], AF.Exp, bias=nmax[:, 0:1],
                                     accum_out=ssum[:])
                rcp = sp.tile([2, 1], F32, tag="rc" + nm)
                nc.vector.reciprocal(rcp[:], ssum[:])
                sm = sp.tile([2, width], F32, tag="sm" + nm)
                nc.vector.tensor_scalar_mul(sm[:], ex[:], rcp[:, 0:1])
                nc.sync.dma_start(out_d[:], sm[:])

    nc.compile()
    return nc


_PROGRAM = None


def kernel(**inputs):
    global _PROGRAM
    if _PROGRAM is None:
        _PROGRAM = build_program()
    nc = _PROGRAM

    w = _pack_weights(inputs)
    items = _item_list()
    x = np.asarray(inputs["inputs"], np.float32)

    in_maps = []
    for c in range(N_CORES):
        xi = np.zeros((SLOTS, 8, 81, FR56), BF16_NP)
        for i in range(CNT[c]):
            b, t0 = items[OFFS[c] + i]
            xi[i] = _build_im2col(x[b, t0:t0 + WIN])
        m = {"x_im2col": xi, "zeros": np.zeros((128, FR56), BF16_NP)}
        for nm, shape, dt in WEIGHT_SPECS:
            npdt = BF16_NP if dt == BF16 else np.float32
            m[nm] = np.ascontiguousarray(w[nm].reshape(shape).astype(npdt))
        in_maps.append(m)

    res = bass_utils.run_bass_kernel_spmd(nc, in_maps,
                                          core_ids=list(range(N_CORES)))
    r0 = res.results[0]
    kernel.last_debug = {k: np.asarray(v) for k, v in r0.items()}
    return np.asarray(r0["throw"], np.float32), np.asarray(r0["tori"], np.float32)
